# revision 3
# baseline (speedup 1.0000x reference)
"""AddRelativePositionalEmbedding Trainium2 kernel.

Per-core problem (B=8 sharded 1 batch-head per core):
  out[r, k1*64+k2] = attn[r, k1*64+k2] + rel_h[r, k1] + rel_w[r, k2]
  rel_h[(h,w), k1] = sum_c q[(h,w),c] * rel_pos_h[h-k1+63, c]
  rel_w[(h,w), k2] = sum_c q[(h,w),c] * rel_pos_w[w-k2+63, c]

Memory-bound: 64MB in + 64MB out per core. TensorE does the tiny einsums,
DVE does two broadcast-adds per streamed 128x4096 tile.
"""

import sys

if "/opt/trn_rl_repo" not in sys.path:
    sys.path.insert(0, "/opt/trn_rl_repo")

import numpy as np

import concourse.bass as bass
import concourse.tile as tile
from concourse import bacc, mybir
from concourse.bass import AP
from concourse.bass_utils import run_bass_kernel_spmd
from concourse.masks import make_identity

F32 = mybir.dt.float32
N_CORES = 8
QH = QW = KH = KW = 64
C = 64
NQ = QH * QW          # 4096 query positions per core
NK = KH * KW          # 4096 key positions
P = 128               # partitions per tile
NCHUNK = NQ // P      # 32 chunks of 128 query rows


def _ap(base: AP, extra_offset: int, dims: list[list[int]]) -> AP:
    """Build a raw AP on base's tensor at base.offset + extra_offset."""
    return AP(base.tensor, base.offset + extra_offset, [list(d) for d in dims])


def build_kernel_body(tc, attn_d: AP, q_d: AP, rph_d: AP, rpw_d: AP, out_d: AP):
    nc = tc.nc
    import contextlib

    ctx = contextlib.ExitStack()
    with ctx:
        consts = ctx.enter_context(tc.tile_pool(name="consts", bufs=1))
        ps_pool = ctx.enter_context(tc.tile_pool(name="psum", bufs=4, space="PSUM"))
        stream = ctx.enter_context(tc.tile_pool(name="stream", bufs=4))

        # ---------------- Phase A: rel_h / rel_w (tiny einsums) ------------
        # q natural layout: q_nat[p, i*64 + c] = q[i*128 + p, c]
        q_nat = consts.tile([P, NCHUNK * C], F32)
        nc.sync.dma_start(
            q_nat[:].rearrange("p (i c) -> p i c", c=C),
            q_d.rearrange("(i p) c -> p i c", p=P),
        )

        ident = consts.tile([P, P], F32)
        make_identity(nc, ident[:])

        # qT[c, r] via PE transpose of each [128, 64] chunk
        qT = consts.tile([C, NQ], F32)
        for i in range(NCHUNK):
            ps_t = ps_pool.tile([C, P], F32, tag="ps_t")
            nc.tensor.transpose(ps_t[:], q_nat[:, i * C:(i + 1) * C], ident[:])
            nc.vector.tensor_copy(out=qT[:, i * P:(i + 1) * P], in_=ps_t[:])

        # transposed rel-pos tables: [c, idx] (small strided DMA)
        rphT = consts.tile([C, 2 * QH - 1], F32)
        nc.sync.dma_start(rphT[:], rph_d.transpose([1, 0]))
        rpwT = consts.tile([C, 2 * QW - 1], F32)
        nc.sync.dma_start(rpwT[:], rpw_d.transpose([1, 0]))

        rphT_b = rphT[:]
        rpwT_b = rpwT[:]
        qT_b = qT[:]
        qT_pitch = qT_b.ap[0][0]

        # stagingH[w, h*64 + k1] = rel_h[(h,w), k1]
        stagingH = consts.tile([QW, QH * KH], F32)
        # stagingW[h, w*64 + k2] = rel_w[(h,w), k2]
        stagingW = consts.tile([QH, QW * KW], F32)

        for h in range(QH):
            ps = ps_pool.tile([QW, KH], F32, tag="ps_mm")
            # out[w, k1] = sum_c qT[c, h*64+w] * rel_pos_hT[c, h+63-k1]
            rhs = _ap(rphT_b, h + KH - 1, [[rphT_b.ap[0][0], C], [-1, KH]])
            nc.tensor.matmul(ps[:], qT_b[:, h * QW:(h + 1) * QW], rhs,
                             start=True, stop=True)
            nc.vector.tensor_copy(out=stagingH[:, h * KH:(h + 1) * KH], in_=ps[:])

        for w in range(QW):
            ps = ps_pool.tile([QH, KW], F32, tag="ps_mm")
            # out[h, k2] = sum_c qT[c, h*64+w] * rel_pos_wT[c, w+63-k2]
            lhsT = _ap(qT_b, w, [[qT_pitch, C], [QW, QH]])
            rhs = _ap(rpwT_b, w + KW - 1, [[rpwT_b.ap[0][0], C], [-1, KW]])
            nc.tensor.matmul(ps[:], lhsT, rhs, start=True, stop=True)
            nc.vector.tensor_copy(out=stagingW[:, w * KW:(w + 1) * KW], in_=ps[:])

        # Scatter into row-chunk layout:
        #   rel_h_sb[(h%2)*64 + w, (h//2)*64 + k1]
        #   rel_w_sb[(h%2)*64 + w, (h//2)*64 + k2]
        rel_h_sb = consts.tile([P, NCHUNK * KH], F32)
        rel_w_sb = consts.tile([P, NCHUNK * KW], F32)
        sh = stagingH[:]
        sw = stagingW[:]
        rh = rel_h_sb[:]
        rw = rel_w_sb[:]
        shp = sh.ap[0][0]   # stagingH partition pitch (elements)
        swp = sw.ap[0][0]
        rhp = rh.ap[0][0]
        rwp = rw.ap[0][0]
        # iterate (w[64], h2[32], k[64]) per h1 in {0, 1} (DMA APs cap at 3 dims)
        for h1 in range(2):
            # src stagingH[w, (h1+2*h2)*64 + k1]: partition dim leads on both
            # sides, legal as a direct sb->sb DMA.
            nc.sync.dma_start(
                _ap(rh, h1 * 64 * rhp, [[rhp, QW], [KH, NCHUNK], [1, KH]]),
                _ap(sh, h1 * KH, [[shp, QW], [2 * KH, NCHUNK], [1, KH]]),
            )
        # rel_w needs a partition<->free shuffle (src partition dim h maps to
        # dst free dim, src free w maps to dst partition) — SBUF APs can't
        # cross partitions in a non-leading dim, so bounce through DRAM where
        # APs are purely linear.
        scratchW = nc.dram_tensor("scratch_w", [QH, QW * KW], F32)
        nc.sync.dma_start(scratchW.ap(), sw)
        for h1 in range(2):
            # src dram element (w, h2, k2) at scratch_w[h1 + 2*h2, w*64 + k2]
            nc.sync.dma_start(
                _ap(rw, h1 * 64 * rwp, [[rwp, QW], [KW, NCHUNK], [1, KW]]),
                _ap(scratchW.ap(), h1 * QW * KW,
                    [[KW, QW], [2 * QW * KW, NCHUNK], [1, KW]]),
            )

        # ---------------- Phase B: stream the attention map ----------------
        for i in range(NCHUNK):
            t = stream.tile([P, NK], F32, tag="attn")
            nc.sync.dma_start(t[:], attn_d[i * P:(i + 1) * P, :])
            t3 = t[:].rearrange("p (a b) -> p a b", b=KW)
            relh = _ap(rh, i * KH, [[rhp, P], [1, KH], [0, KW]])
            relw = _ap(rw, i * KW, [[rwp, P], [0, KH], [1, KW]])
            nc.vector.tensor_tensor(out=t3, in0=t3, in1=relh, op=mybir.AluOpType.add)
            nc.vector.tensor_tensor(out=t3, in0=t3, in1=relw, op=mybir.AluOpType.add)
            nc.scalar.dma_start(out_d[i * P:(i + 1) * P, :], t[:])


_NC_CACHE = {}


def build_nc():
    if "nc" in _NC_CACHE:
        return _NC_CACHE["nc"]
    nc = bacc.Bacc("TRN2", target_bir_lowering=False, debug=False,
                   num_devices=N_CORES)
    attn = nc.dram_tensor("attention_map", [NQ, NK], F32, kind="ExternalInput")
    q = nc.dram_tensor("queries", [NQ, C], F32, kind="ExternalInput")
    rph = nc.dram_tensor("rel_pos_h", [2 * QH - 1, C], F32, kind="ExternalInput")
    rpw = nc.dram_tensor("rel_pos_w", [2 * QW - 1, C], F32, kind="ExternalInput")
    out = nc.dram_tensor("out", [NQ, NK], F32, kind="ExternalOutput")
    with tile.TileContext(nc) as tc:
        build_kernel_body(tc, attn.ap(), q.ap(), rph.ap(), rpw.ap(), out.ap())
    nc.compile()
    _NC_CACHE["nc"] = nc
    return nc


def make_in_maps(attention_map, queries, rel_pos_h, rel_pos_w):
    attn = np.ascontiguousarray(np.asarray(attention_map, dtype=np.float32))
    q = np.ascontiguousarray(np.asarray(queries, dtype=np.float32))
    rph = np.ascontiguousarray(np.asarray(rel_pos_h, dtype=np.float32))
    rpw = np.ascontiguousarray(np.asarray(rel_pos_w, dtype=np.float32))
    return [
        {"attention_map": attn[i], "queries": q[i],
         "rel_pos_h": rph, "rel_pos_w": rpw}
        for i in range(N_CORES)
    ]


def kernel(attention_map, queries, rel_pos_h, rel_pos_w,
           query_h=64, query_w=64, key_h=64, key_w=64, **_unused):
    nc = build_nc()
    in_maps = make_in_maps(attention_map, queries, rel_pos_h, rel_pos_w)
    res = run_bass_kernel_spmd(nc, in_maps, core_ids=list(range(N_CORES)))
    out = np.stack([res.results[i]["out"] for i in range(N_CORES)], axis=0)
    return out
